# revision 5
# baseline (speedup 1.0000x reference)
"""Trainium2 Bass kernel for nn_Attn_24404004176217.

Single-head causal attention block: qkv = x @ Wqkv; attn; out @ Wproj.
Sharding: data-parallel over batch — B=8 batches, one per NeuronCore.
Each core runs an identical NEFF on its own batch slice; weights are
replicated. No collectives.

Per-core algorithm (T=2048, C=1024, all matmuls bf16 with fp32 PSUM
accumulation):
  1. x [T,C] is PE-transposed once into xT [C,T] (bf16).  Every later
     matmul then has its contraction dim on partitions naturally.
  2. qT = (Wq stationary) @ xT, kT likewise; v = (xT stationary) @ Wv
     comes out in natural [s,c] layout.
  3. Attention uses a transposed softmax: scoresT[s,t] = kT.T @ qT per
     (s-chunk, t-block); exp(scale*s) needs no max-subtraction because
     scores ~ N(0,1); causal masking is a 0/1 multiply on the 4 diagonal
     chunk offsets; row sums come from a ones-vector matmul; avT =
     (v stationary) @ pT accumulates unnormalized.
  4. y = (avT stationary) @ Wproj lands in natural [t,c] layout; the
     1/rowsum normalization is folded into the PSUM-evicting activation
     copy as a per-partition scale.
"""

import sys

if "/opt/trn_rl_repo" not in sys.path:
    sys.path.insert(0, "/opt/trn_rl_repo")

import numpy as np

import concourse.bass as bass
import concourse.mybir as mybir
from concourse import bacc
from concourse.bass_utils import run_bass_kernel_spmd
from concourse.masks import make_identity
from concourse.tile import TileContext

P = 128
FP32 = mybir.dt.float32
BF16 = mybir.dt.bfloat16
AF = mybir.ActivationFunctionType
ALU = mybir.AluOpType

N_CORES = 8

LAST_RESULTS = None  # BassKernelResults of the most recent run (for test.py)


def build_attn_nc(T=2048, C=1024, TB=512):
    """Build the single-core Bass module (same NEFF runs SPMD on all cores)."""
    CS = C // P  # contraction subtiles over C
    TS = T // P  # 128-row tiles over T
    NTB = T // TB  # t-blocks for attention
    SCB = TB // P  # s-chunks per t-block diagonal group
    NCO = C // 512  # 512-wide output chunks over C
    NT5 = T // 512  # 512-wide chunks over T
    scale = float(C) ** -0.5

    nc = bacc.Bacc("TRN2", target_bir_lowering=False, debug=False)
    x_d = nc.dram_tensor("x", [T, C], FP32, kind="ExternalInput")
    wqkv_d = nc.dram_tensor("Wqkv", [C, 3 * C], FP32, kind="ExternalInput")
    wproj_d = nc.dram_tensor("Wproj", [C, C], FP32, kind="ExternalInput")
    y_d = nc.dram_tensor("y", [T, C], FP32, kind="ExternalOutput")

    with TileContext(nc) as tc:
        with (
            tc.tile_pool(name="const", bufs=1) as const_pool,
            tc.tile_pool(name="persist", bufs=1) as persist,
        ):
            ident = const_pool.tile([P, P], FP32)
            make_identity(nc, ident)
            ones_col = const_pool.tile([P, 1], BF16)
            nc.gpsimd.memset(ones_col, 1.0)
            one_one = const_pool.tile([1, 1], FP32)
            nc.gpsimd.memset(one_one, 1.0)
            # 0/1 causal masks for the diagonal s-chunks. masks[s, m, t] = 1
            # iff t >= s + m*128 (block-local coords).
            masks = const_pool.tile([P, SCB, TB], BF16)
            nc.gpsimd.memset(masks, 1.0)
            for m in range(SCB):
                nc.gpsimd.affine_select(
                    out=masks[:, m, :],
                    in_=masks[:, m, :],
                    compare_op=ALU.is_ge,
                    fill=0.0,
                    base=-(m * P),
                    channel_multiplier=-1,
                    pattern=[[1, TB]],
                )

            # Persistent across phases
            kT_bf = persist.tile([P, CS, T], BF16)  # kT[c, s]
            v_bf = persist.tile([P, TS, C], BF16)  # v[s, c]
            qT_bf = persist.tile([P, CS, T], BF16)  # qT[c, t]
            wproj_bf = persist.tile([P, CS, C], BF16)

            # ---------------- Phase A: load x, transpose, build q/k/v ----
            with (
                tc.tile_pool(name="xT", bufs=1) as xT_pool,
                tc.tile_pool(name="xc", bufs=3) as x_pool,
                tc.tile_pool(name="wbf", bufs=2) as wbf_pool,
                tc.tile_pool(name="wst", bufs=2) as wstage_pool,
                tc.tile_pool(name="trps", bufs=2, space="PSUM") as tr_psum,
                tc.tile_pool(name="qkvps", bufs=4, space="PSUM") as qkv_psum,
            ):

                def load_w_bf(dst_bf, src_ap):
                    # src_ap: [C, C] DRAM slice; dst layout [p, cs, n]
                    for cs in range(CS):
                        wst = wstage_pool.tile([P, C], FP32, tag="wst")
                        nc.gpsimd.dma_start(wst, src_ap[cs * P : (cs + 1) * P, :])
                        nc.vector.tensor_copy(dst_bf[:, cs, :], wst)

                wq_bf = wbf_pool.tile([P, CS, C], BF16, tag="w")
                load_w_bf(wq_bf, wqkv_d[:, 0:C])

                xT_bf = xT_pool.tile([P, CS, T], BF16)
                for ts in range(TS):
                    xc = x_pool.tile([P, C], FP32, tag="xc")
                    nc.gpsimd.dma_start(xc, x_d[ts * P : (ts + 1) * P, :])
                    for cs in range(CS):
                        pt = tr_psum.tile([P, P], FP32, tag="tr")
                        nc.tensor.transpose(pt, xc[:, cs * P : (cs + 1) * P], ident)
                        nc.vector.tensor_copy(
                            xT_bf[:, cs, ts * P : (ts + 1) * P], pt
                        )

                # qT = Wq-stationary @ xT-moving
                def qk_phase(w_bf, out_bf):
                    for cq in range(CS):
                        for tch in range(NT5):
                            pq = qkv_psum.tile([P, 512], FP32, tag="qkv")
                            for cs in range(CS):
                                nc.tensor.matmul(
                                    pq,
                                    lhsT=w_bf[:, cs, cq * P : (cq + 1) * P],
                                    rhs=xT_bf[:, cs, tch * 512 : (tch + 1) * 512],
                                    start=(cs == 0),
                                    stop=(cs == CS - 1),
                                )
                            nc.vector.tensor_copy(
                                out_bf[:, cq, tch * 512 : (tch + 1) * 512], pq
                            )

                wk_bf = wbf_pool.tile([P, CS, C], BF16, tag="w")
                load_w_bf(wk_bf, wqkv_d[:, C : 2 * C])
                qk_phase(wq_bf, qT_bf)

                wv_bf = wbf_pool.tile([P, CS, C], BF16, tag="w")
                load_w_bf(wv_bf, wqkv_d[:, 2 * C : 3 * C])
                qk_phase(wk_bf, kT_bf)

                load_w_bf(wproj_bf, wproj_d)

                # v = xT-stationary @ Wv-moving → natural [s, c]
                for ss in range(TS):
                    for cv in range(NCO):
                        pv = qkv_psum.tile([P, 512], FP32, tag="qkv")
                        for cs in range(CS):
                            nc.tensor.matmul(
                                pv,
                                lhsT=xT_bf[:, cs, ss * P : (ss + 1) * P],
                                rhs=wv_bf[:, cs, cv * 512 : (cv + 1) * 512],
                                start=(cs == 0),
                                stop=(cs == CS - 1),
                            )
                        nc.vector.tensor_copy(
                            v_bf[:, ss, cv * 512 : (cv + 1) * 512], pv
                        )

            # ---------------- Phase B: attention + proj per t-block ------
            with (
                tc.tile_pool(name="pT", bufs=2) as pT_pool,
                tc.tile_pool(name="avT", bufs=2) as avT_pool,
                tc.tile_pool(name="ysb", bufs=3) as y_pool,
                tc.tile_pool(name="small", bufs=8) as small_pool,
                tc.tile_pool(name="scps", bufs=2, space="PSUM") as scores_psum,
                tc.tile_pool(name="avps", bufs=3, space="PSUM") as av_psum,
                tc.tile_pool(name="smps", bufs=1, space="PSUM") as sums_psum,
                tc.tile_pool(name="invps", bufs=1, space="PSUM") as inv_psum,
            ):
                for tb in range(NTB):
                    nsc = SCB * (tb + 1)
                    t0 = tb * TB

                    # scoresT + exp → pT (bf16), causal-masked on diagonal
                    pT = pT_pool.tile([P, TS, TB], BF16, tag="pT")
                    for sc in range(nsc):
                        ps = scores_psum.tile([P, TB], FP32, tag="sc")
                        for cs in range(CS):
                            nc.tensor.matmul(
                                ps,
                                lhsT=kT_bf[:, cs, sc * P : (sc + 1) * P],
                                rhs=qT_bf[:, cs, t0 : t0 + TB],
                                start=(cs == 0),
                                stop=(cs == CS - 1),
                            )
                        nc.scalar.activation(pT[:, sc, :], ps, AF.Exp, scale=scale)
                        m = sc - SCB * tb
                        if m >= 0:  # diagonal chunk → apply triangular mask
                            nc.vector.tensor_mul(
                                pT[:, sc, :], pT[:, sc, :], masks[:, m, :]
                            )

                    # row sums (per t column) via ones-vector matmul
                    psums = sums_psum.tile([1, TB], FP32, tag="sums")
                    for sc in range(nsc):
                        nc.tensor.matmul(
                            psums,
                            lhsT=ones_col,
                            rhs=pT[:, sc, :],
                            start=(sc == 0),
                            stop=(sc == nsc - 1),
                        )

                    # avT[c, t] = v-stationary @ pT-moving (unnormalized)
                    avT = avT_pool.tile([P, CS, TB], BF16, tag="avT")
                    for cs in range(CS):
                        pav = av_psum.tile([P, TB], FP32, tag="av")
                        for sc in range(nsc):
                            nc.tensor.matmul(
                                pav,
                                lhsT=v_bf[:, sc, cs * P : (cs + 1) * P],
                                rhs=pT[:, sc, :],
                                start=(sc == 0),
                                stop=(sc == nsc - 1),
                            )
                        nc.vector.tensor_copy(avT[:, cs, :], pav)

                    # 1/rowsum, transposed into per-partition [128, 1] tiles
                    inv_row = small_pool.tile([1, TB], FP32, tag="inv_row")
                    nc.vector.reciprocal(inv_row, psums)
                    inv_sbs = []
                    for ti in range(TB // P):
                        pinv = inv_psum.tile([P, 1], FP32, tag="pinv")
                        nc.tensor.transpose(
                            pinv, inv_row[0:1, ti * P : (ti + 1) * P], one_one
                        )
                        inv_sb = small_pool.tile([P, 1], FP32, tag="inv_sb")
                        nc.vector.tensor_copy(inv_sb, pinv)
                        inv_sbs.append(inv_sb)

                    # proj: y[t, co] = avT-stationary @ Wproj-moving,
                    # normalization folded into the PSUM-evicting copy
                    for ti in range(TB // P):
                        for co in range(NCO):
                            pproj = av_psum.tile([P, 512], FP32, tag="av")
                            for cs in range(CS):
                                nc.tensor.matmul(
                                    pproj,
                                    lhsT=avT[:, cs, ti * P : (ti + 1) * P],
                                    rhs=wproj_bf[:, cs, co * 512 : (co + 1) * 512],
                                    start=(cs == 0),
                                    stop=(cs == CS - 1),
                                )
                            ysb = y_pool.tile([P, 512], FP32, tag="ysb")
                            nc.scalar.activation(
                                ysb, pproj, AF.Copy, scale=inv_sbs[ti]
                            )
                            nc.gpsimd.dma_start(
                                y_d[
                                    t0 + ti * P : t0 + (ti + 1) * P,
                                    co * 512 : (co + 1) * 512,
                                ],
                                ysb,
                            )

    nc.compile()
    return nc


_NC_CACHE = {}


def _get_nc(T, C, TB):
    key = (T, C, TB)
    if key not in _NC_CACHE:
        _NC_CACHE[key] = build_attn_nc(T, C, TB)
    return _NC_CACHE[key]


def kernel(x: np.ndarray, Wqkv: np.ndarray, Wproj: np.ndarray, **kw) -> np.ndarray:
    global LAST_RESULTS
    B, T, C = x.shape
    assert B == N_CORES, f"expected B == {N_CORES}, got {B}"
    nc = _get_nc(T, C, 512 if T % 512 == 0 else 256)

    x = np.ascontiguousarray(x, dtype=np.float32)
    Wqkv = np.ascontiguousarray(Wqkv, dtype=np.float32)
    Wproj = np.ascontiguousarray(Wproj, dtype=np.float32)

    in_maps = [
        {"x": x[b], "Wqkv": Wqkv, "Wproj": Wproj} for b in range(N_CORES)
    ]
    res = run_bass_kernel_spmd(
        nc,
        in_maps,
        core_ids=list(range(N_CORES)),
        trace=bool(int(__import__("os").environ.get("ATTN_TRACE", "0"))),
    )
    LAST_RESULTS = res
    return np.stack([res.results[b]["y"] for b in range(N_CORES)], axis=0)


# revision 6
# speedup vs baseline: 2876.6814x; 2876.6814x over previous
"""Trainium2 Bass kernel for nn_Attn_24404004176217.

Single-head causal attention block: qkv = x @ Wqkv; attn; out @ Wproj.
Sharding: data-parallel over batch — B=8 batches, one per NeuronCore.
Each core runs an identical NEFF on its own batch slice; weights are
replicated. No collectives.

Per-core algorithm (T=2048, C=1024, all matmuls bf16 with fp32 PSUM
accumulation):
  1. x [T,C] is PE-transposed once into xT [C,T] (bf16).  Every later
     matmul then has its contraction dim on partitions naturally.
  2. qT = (Wq stationary) @ xT, kT likewise; v = (xT stationary) @ Wv
     comes out in natural [s,c] layout.
  3. Attention uses a transposed softmax: scoresT[s,t] = kT.T @ qT per
     (s-chunk, t-block); exp(scale*s) needs no max-subtraction because
     scores ~ N(0,1); causal masking is a 0/1 multiply on the 4 diagonal
     chunk offsets; row sums come from a ones-vector matmul; avT =
     (v stationary) @ pT accumulates unnormalized.
  4. y = (avT stationary) @ Wproj lands in natural [t,c] layout; the
     1/rowsum normalization is folded into the PSUM-evicting activation
     copy as a per-partition scale.

`repeat` emits the whole program K times into one NEFF (benchmarking:
the t(K)-t(1) slope isolates pure device time from dispatch overhead).
"""

import os
import sys

if "/opt/trn_rl_repo" not in sys.path:
    sys.path.insert(0, "/opt/trn_rl_repo")

import numpy as np

import concourse.bass as bass
import concourse.mybir as mybir
from concourse import bacc
from concourse.bass_utils import run_bass_kernel_spmd
from concourse.masks import make_identity
from concourse.tile import TileContext

P = 128
FP32 = mybir.dt.float32
BF16 = mybir.dt.bfloat16
AF = mybir.ActivationFunctionType
ALU = mybir.AluOpType

N_CORES = 8

LAST_RESULTS = None  # BassKernelResults of the most recent run (for test.py)


def build_attn_nc(T=2048, C=1024, TB=512, repeat=1):
    """Build the single-core Bass module (same NEFF runs SPMD on all cores)."""
    CS = C // P  # contraction subtiles over C
    TS = T // P  # 128-row tiles over T
    NTB = T // TB  # t-blocks for attention
    SCB = TB // P  # s-chunks per t-block diagonal group
    NCO = C // 512  # 512-wide output chunks over C
    NT5 = T // 512  # 512-wide chunks over T
    scale = float(C) ** -0.5

    nc = bacc.Bacc("TRN2", target_bir_lowering=False, debug=False)
    x_d = nc.dram_tensor("x", [T, C], FP32, kind="ExternalInput")
    wqkv_d = nc.dram_tensor("Wqkv", [C, 3 * C], FP32, kind="ExternalInput")
    wproj_d = nc.dram_tensor("Wproj", [C, C], FP32, kind="ExternalInput")
    y_d = nc.dram_tensor("y", [T, C], FP32, kind="ExternalOutput")

    with TileContext(nc) as tc:
        with tc.tile_pool(name="const", bufs=1) as const_pool:
            ident = const_pool.tile([P, P], FP32)
            make_identity(nc, ident)
            ones_col = const_pool.tile([P, 1], BF16)
            nc.gpsimd.memset(ones_col, 1.0)
            one_one = const_pool.tile([1, 1], FP32)
            nc.gpsimd.memset(one_one, 1.0)
            # 0/1 causal masks for the diagonal s-chunks. masks[s, m, t] = 1
            # iff t >= s + m*128 (block-local coords).
            masks = const_pool.tile([P, SCB, TB], BF16)
            nc.gpsimd.memset(masks, 1.0)
            for m in range(SCB):
                nc.gpsimd.affine_select(
                    out=masks[:, m, :],
                    in_=masks[:, m, :],
                    compare_op=ALU.is_ge,
                    fill=0.0,
                    base=-(m * P),
                    channel_multiplier=-1,
                    pattern=[[1, TB]],
                )

            for _rep in range(repeat):
                _emit_one(
                    nc, tc, _rep,
                    x_d, wqkv_d, wproj_d, y_d,
                    ident, ones_col, one_one, masks,
                    T, C, TB, CS, TS, NTB, SCB, NCO, NT5, scale,
                )

    nc.compile()
    return nc


def _emit_one(
    nc, tc, rep,
    x_d, wqkv_d, wproj_d, y_d,
    ident, ones_col, one_one, masks,
    T, C, TB, CS, TS, NTB, SCB, NCO, NT5, scale,
):
    with tc.tile_pool(name=f"persist{rep}", bufs=1) as persist:
        # Persistent across phases (within one repetition)
        kT_bf = persist.tile([P, CS, T], BF16, name=f"kT{rep}")  # kT[c, s]
        v_bf = persist.tile([P, TS, C], BF16, name=f"v{rep}")  # v[s, c]
        qT_bf = persist.tile([P, CS, T], BF16, name=f"qT{rep}")  # qT[c, t]
        wproj_bf = persist.tile([P, CS, C], BF16, name=f"wp{rep}")

        # ---------------- Phase A: load x, transpose, build q/k/v ----
        with (
            tc.tile_pool(name=f"xT{rep}", bufs=1) as xT_pool,
            tc.tile_pool(name=f"xc{rep}", bufs=3) as x_pool,
            tc.tile_pool(name=f"wbf{rep}", bufs=2) as wbf_pool,
            tc.tile_pool(name=f"wst{rep}", bufs=2) as wstage_pool,
            tc.tile_pool(name=f"trps{rep}", bufs=2, space="PSUM") as tr_psum,
            tc.tile_pool(name=f"qkvps{rep}", bufs=4, space="PSUM") as qkv_psum,
        ):

            def load_w_bf(dst_bf, src_ap):
                # src_ap: [C, C] DRAM slice; dst layout [p, cs, n]
                for cs in range(CS):
                    wst = wstage_pool.tile([P, C], FP32, tag="wst", name=f"wst{rep}")
                    nc.gpsimd.dma_start(wst, src_ap[cs * P : (cs + 1) * P, :])
                    nc.vector.tensor_copy(dst_bf[:, cs, :], wst)

            wq_bf = wbf_pool.tile([P, CS, C], BF16, tag="w", name=f"wq{rep}")
            load_w_bf(wq_bf, wqkv_d[:, 0:C])

            xT_bf = xT_pool.tile([P, CS, T], BF16, name=f"xT{rep}")
            for ts in range(TS):
                xc = x_pool.tile([P, C], FP32, tag="xc", name=f"xc{rep}")
                nc.gpsimd.dma_start(xc, x_d[ts * P : (ts + 1) * P, :])
                for cs in range(CS):
                    pt = tr_psum.tile([P, P], FP32, tag="tr", name=f"tr{rep}")
                    nc.tensor.transpose(pt, xc[:, cs * P : (cs + 1) * P], ident)
                    nc.vector.tensor_copy(xT_bf[:, cs, ts * P : (ts + 1) * P], pt)

            # qT = Wq-stationary @ xT-moving
            def qk_phase(w_bf, out_bf):
                for cq in range(CS):
                    for tch in range(NT5):
                        pq = qkv_psum.tile(
                            [P, 512], FP32, tag="qkv", name=f"pqkv{rep}"
                        )
                        for cs in range(CS):
                            nc.tensor.matmul(
                                pq,
                                lhsT=w_bf[:, cs, cq * P : (cq + 1) * P],
                                rhs=xT_bf[:, cs, tch * 512 : (tch + 1) * 512],
                                start=(cs == 0),
                                stop=(cs == CS - 1),
                            )
                        nc.vector.tensor_copy(
                            out_bf[:, cq, tch * 512 : (tch + 1) * 512], pq
                        )

            wk_bf = wbf_pool.tile([P, CS, C], BF16, tag="w", name=f"wk{rep}")
            load_w_bf(wk_bf, wqkv_d[:, C : 2 * C])
            qk_phase(wq_bf, qT_bf)

            wv_bf = wbf_pool.tile([P, CS, C], BF16, tag="w", name=f"wv{rep}")
            load_w_bf(wv_bf, wqkv_d[:, 2 * C : 3 * C])
            qk_phase(wk_bf, kT_bf)

            load_w_bf(wproj_bf, wproj_d)

            # v = xT-stationary @ Wv-moving → natural [s, c]
            for ss in range(TS):
                for cv in range(NCO):
                    pv = qkv_psum.tile([P, 512], FP32, tag="qkv", name=f"pqkv{rep}")
                    for cs in range(CS):
                        nc.tensor.matmul(
                            pv,
                            lhsT=xT_bf[:, cs, ss * P : (ss + 1) * P],
                            rhs=wv_bf[:, cs, cv * 512 : (cv + 1) * 512],
                            start=(cs == 0),
                            stop=(cs == CS - 1),
                        )
                    nc.vector.tensor_copy(
                        v_bf[:, ss, cv * 512 : (cv + 1) * 512], pv
                    )

        # ---------------- Phase B: attention + proj per t-block ------
        with (
            tc.tile_pool(name=f"pT{rep}", bufs=2) as pT_pool,
            tc.tile_pool(name=f"avT{rep}", bufs=2) as avT_pool,
            tc.tile_pool(name=f"ysb{rep}", bufs=3) as y_pool,
            tc.tile_pool(name=f"small{rep}", bufs=8) as small_pool,
            tc.tile_pool(name=f"scps{rep}", bufs=2, space="PSUM") as scores_psum,
            tc.tile_pool(name=f"avps{rep}", bufs=3, space="PSUM") as av_psum,
            tc.tile_pool(name=f"smps{rep}", bufs=1, space="PSUM") as sums_psum,
            tc.tile_pool(name=f"invps{rep}", bufs=1, space="PSUM") as inv_psum,
        ):
            for tb in range(NTB):
                nsc = SCB * (tb + 1)
                t0 = tb * TB

                # scoresT + exp → pT (bf16), causal-masked on diagonal
                pT = pT_pool.tile([P, TS, TB], BF16, tag="pT", name=f"pT{rep}")
                for sc in range(nsc):
                    ps = scores_psum.tile([P, TB], FP32, tag="sc", name=f"psc{rep}")
                    for cs in range(CS):
                        nc.tensor.matmul(
                            ps,
                            lhsT=kT_bf[:, cs, sc * P : (sc + 1) * P],
                            rhs=qT_bf[:, cs, t0 : t0 + TB],
                            start=(cs == 0),
                            stop=(cs == CS - 1),
                        )
                    nc.scalar.activation(pT[:, sc, :], ps, AF.Exp, scale=scale)
                    m = sc - SCB * tb
                    if m >= 0:  # diagonal chunk → apply triangular mask
                        nc.vector.tensor_mul(
                            pT[:, sc, :], pT[:, sc, :], masks[:, m, :]
                        )

                # row sums (per t column) via ones-vector matmul
                psums = sums_psum.tile([1, TB], FP32, tag="sums", name=f"psm{rep}")
                for sc in range(nsc):
                    nc.tensor.matmul(
                        psums,
                        lhsT=ones_col,
                        rhs=pT[:, sc, :],
                        start=(sc == 0),
                        stop=(sc == nsc - 1),
                    )

                # avT[c, t] = v-stationary @ pT-moving (unnormalized)
                avT = avT_pool.tile([P, CS, TB], BF16, tag="avT", name=f"avT{rep}")
                for cs in range(CS):
                    pav = av_psum.tile([P, TB], FP32, tag="av", name=f"pav{rep}")
                    for sc in range(nsc):
                        nc.tensor.matmul(
                            pav,
                            lhsT=v_bf[:, sc, cs * P : (cs + 1) * P],
                            rhs=pT[:, sc, :],
                            start=(sc == 0),
                            stop=(sc == nsc - 1),
                        )
                    nc.vector.tensor_copy(avT[:, cs, :], pav)

                # 1/rowsum, transposed into per-partition [128, 1] tiles
                inv_row = small_pool.tile(
                    [1, TB], FP32, tag="inv_row", name=f"ivr{rep}"
                )
                nc.vector.reciprocal(inv_row, psums)
                inv_sbs = []
                for ti in range(TB // P):
                    pinv = inv_psum.tile([P, 1], FP32, tag="pinv", name=f"piv{rep}")
                    nc.tensor.transpose(
                        pinv, inv_row[0:1, ti * P : (ti + 1) * P], one_one
                    )
                    inv_sb = small_pool.tile(
                        [P, 1], FP32, tag="inv_sb", name=f"ivs{rep}"
                    )
                    nc.vector.tensor_copy(inv_sb, pinv)
                    inv_sbs.append(inv_sb)

                # proj: y[t, co] = avT-stationary @ Wproj-moving,
                # normalization folded into the PSUM-evicting copy
                for ti in range(TB // P):
                    for co in range(NCO):
                        pproj = av_psum.tile(
                            [P, 512], FP32, tag="av", name=f"pav{rep}"
                        )
                        for cs in range(CS):
                            nc.tensor.matmul(
                                pproj,
                                lhsT=avT[:, cs, ti * P : (ti + 1) * P],
                                rhs=wproj_bf[:, cs, co * 512 : (co + 1) * 512],
                                start=(cs == 0),
                                stop=(cs == CS - 1),
                            )
                        ysb = y_pool.tile([P, 512], FP32, tag="ysb", name=f"ysb{rep}")
                        nc.scalar.activation(ysb, pproj, AF.Copy, scale=inv_sbs[ti])
                        nc.gpsimd.dma_start(
                            y_d[
                                t0 + ti * P : t0 + (ti + 1) * P,
                                co * 512 : (co + 1) * 512,
                            ],
                            ysb,
                        )


_NC_CACHE = {}


def _get_nc(T, C, TB, repeat=1):
    key = (T, C, TB, repeat)
    if key not in _NC_CACHE:
        _NC_CACHE[key] = build_attn_nc(T, C, TB, repeat)
    return _NC_CACHE[key]


def kernel(x: np.ndarray, Wqkv: np.ndarray, Wproj: np.ndarray, **kw) -> np.ndarray:
    global LAST_RESULTS
    B, T, C = x.shape
    assert B == N_CORES, f"expected B == {N_CORES}, got {B}"
    nc = _get_nc(T, C, 512 if T % 512 == 0 else 256)

    x = np.ascontiguousarray(x, dtype=np.float32)
    Wqkv = np.ascontiguousarray(Wqkv, dtype=np.float32)
    Wproj = np.ascontiguousarray(Wproj, dtype=np.float32)

    in_maps = [
        {"x": x[b], "Wqkv": Wqkv, "Wproj": Wproj} for b in range(N_CORES)
    ]
    res = run_bass_kernel_spmd(
        nc,
        in_maps,
        core_ids=list(range(N_CORES)),
        trace=bool(int(os.environ.get("ATTN_TRACE", "0"))),
    )
    LAST_RESULTS = res
    return np.stack([res.results[b]["y"] for b in range(N_CORES)], axis=0)
